# revision 28
# baseline (speedup 1.0000x reference)
"""Multi-head causal self-attention on 8 Trainium2 NeuronCores.

Problem: B=2, S=2048, E=1024, H=16 heads (D=64), causal mask, f32 I/O.

Sharding: (batch x head-group) -> 8 cores, as in the r* baseline (column-
parallel QKV, local attention, row-parallel partial output projection,
host-side partial sum + bias fold).

v5 restructure (exp-bound schedule):
  - Trace analysis showed attention is ScalarE(exp)-bound (~1.15us/iter vs
    PE ~0.65us) and the exp stream only started at 64us because Q,V,K
    projections fully serialize ahead of it on the in-order PE queue.
  - New DMA order xq, xk, xv: attention pair-0 exp stream starts right
    after the K m=0 projection (~34us), bounded by the 9MB q+k input DMA.
  - All non-critical PE work (K m=1 projection, V-projection chains, AV
    matmuls, output projection) is deferred into a budgeted pop queue that
    fills the ~0.5us/iter PE slack of the ACT-bound attention loop.
    Deferred AVs buffer their attn tiles in SBUF (at pool bufs=26).
  - QK is emitted as two concurrent row-tiled K=64 matmuls (even head in
    partitions 0:64, odd in 64:128 - layout already provided it); halves
    QK PE time and removes the need to zero-pad qh/kh.
  - qh/kh live as per-(m, superblock) tiles so the first QK only waits on
    the first K-chain bias-add, not the whole projection.
  - pop items carry (cost, min_iter): min_iter keeps a pop whose deps lie
    in the future out of the PE queue (head-of-line), cost keeps the
    per-iteration emitted PE work under the exp period so QK(i+1) never
    drifts behind and ScalarE never starves.
"""

import os
import sys

for _p in ("/opt/trn_rl_repo",):
    if _p not in sys.path and os.path.isdir(_p):
        sys.path.insert(0, _p)

import numpy as np
import ml_dtypes

import concourse.bacc as bacc
from concourse import mybir
from concourse.tile import TileContext
from concourse.bass_utils import run_bass_kernel_spmd

BF16 = ml_dtypes.bfloat16
P = 128
B, S, E, H, D = 2, 2048, 1024, 16, 64
HPC = 4            # heads per core
DC = HPC * D       # 256 output dims per core per projection
NCORES = 8
QSUP = 512         # q-superblock (matmul free dim)
NSUP = S // QSUP   # 4
NKB = S // P       # 16 k-blocks
SCALE = float(np.sqrt(D))
LOG2E = float(np.log2(np.e))
LN2 = float(np.log(2.0))

AF = mybir.ActivationFunctionType
f32 = mybir.dt.float32
bf16 = mybir.dt.bfloat16

_CACHE = {}
LAST = {}


def _install_axon_profile_shim():
    """Provide antenv.axon_hooks (absent in this image) so
    run_bass_kernel_spmd(trace=True) can NTFF-profile via libaxon_pjrt.so."""
    try:
        import antenv.axon_hooks  # noqa: F401
        return
    except ImportError:
        pass
    import contextlib
    import ctypes
    import types

    import antenv

    state = {"hook": None, "tried": False}

    def _build_hook():
        so_path = "/opt/axon/libaxon_pjrt.so"
        if not os.path.exists(so_path):
            return None
        lib = ctypes.CDLL(so_path)
        if not hasattr(lib, "axon_start_nrt_profile"):
            return None
        lib.axon_start_nrt_profile.argtypes = [
            ctypes.POINTER(ctypes.c_int64),
            ctypes.c_size_t,
        ]
        lib.axon_start_nrt_profile.restype = ctypes.c_int64
        lib.axon_stop_nrt_profile.argtypes = [ctypes.c_char_p]
        lib.axon_stop_nrt_profile.restype = ctypes.c_int64

        @contextlib.contextmanager
        def _hook(output_dir, device_ids):
            import jax

            jax.devices()
            if device_ids:
                ids = (ctypes.c_int64 * len(device_ids))(*device_ids)
                rc = lib.axon_start_nrt_profile(ids, len(device_ids))
            else:
                rc = lib.axon_start_nrt_profile(None, 0)
            if rc != 0:
                raise RuntimeError(f"axon_start_nrt_profile rc={rc}")
            try:
                yield
            finally:
                n = lib.axon_stop_nrt_profile(str(output_dir).encode())
                if n < 0:
                    raise RuntimeError(f"axon_stop_nrt_profile rc={n}")
                print(f"profile: {n} file(s) written to {output_dir}")

        return _hook

    mod = types.ModuleType("antenv.axon_hooks")

    def set_axon_ntff_profile_hook(h):
        state["hook"] = h
        state["tried"] = True

    def get_axon_ntff_profile_hook():
        if not state["tried"]:
            state["hook"] = _build_hook()
            state["tried"] = True
        return state["hook"]

    mod.set_axon_ntff_profile_hook = set_axon_ntff_profile_hook
    mod.get_axon_ntff_profile_hook = get_axon_ntff_profile_hook
    sys.modules["antenv.axon_hooks"] = mod
    antenv.axon_hooks = mod


_install_axon_profile_shim()


def _build_nc(causal: bool):
    nc = bacc.Bacc(None, target_bir_lowering=False)

    xqT = nc.dram_tensor("xqT", [E, S], bf16, kind="ExternalInput")
    xkT = nc.dram_tensor("xkT", [E, S], bf16, kind="ExternalInput")
    xvT = nc.dram_tensor("xvT", [E, S], bf16, kind="ExternalInput")
    wqT = nc.dram_tensor("wqT", [P, 8, DC], bf16, kind="ExternalInput")
    wkT = nc.dram_tensor("wkT", [P, 8, DC], bf16, kind="ExternalInput")
    wvT = nc.dram_tensor("wvT", [P, 8, DC], bf16, kind="ExternalInput")
    woT = nc.dram_tensor("woT", [P, 2, E], bf16, kind="ExternalInput")
    bqk = nc.dram_tensor("bqk", [P, 4], f32, kind="ExternalInput")
    cmask = nc.dram_tensor("cmask", [P, 2, P], bf16, kind="ExternalInput")
    out = nc.dram_tensor("out", [S, E], bf16, kind="ExternalOutput")

    with TileContext(nc) as tc:
        with (
            tc.tile_pool(name="consts", bufs=1) as consts,
            tc.tile_pool(name="xin", bufs=16) as xin,
            tc.tile_pool(name="acts", bufs=1) as acts,
            tc.tile_pool(name="attn", bufs=30) as attn,
            tc.tile_pool(name="norm", bufs=4) as norm,
            tc.tile_pool(name="osb", bufs=3) as osb,
            tc.tile_pool(name="stp", bufs=2, space="PSUM") as stp,
            tc.tile_pool(name="cpool", bufs=2, space="PSUM") as cpool,
        ):
            # ---- input DMAs: q, k first (critical path), v last ------------
            wq_sb = consts.tile([P, 8, DC], bf16)
            wk_sb = consts.tile([P, 8, DC], bf16)
            wv_sb = consts.tile([P, 8, DC], bf16)
            wo_sb = consts.tile([P, 2, E], bf16)
            bqk_sb = consts.tile([P, 4], f32)
            nc.sync.dma_start(wq_sb, wqT[:])
            nc.sync.dma_start(bqk_sb[:], bqk[:])

            def load_x(xT):
                xr = xT.rearrange("(ko p) s -> ko p s", p=P)
                tiles = []
                for ko in range(8):
                    t = xin.tile([P, S], bf16, tag="xin", name=f"x_{xT.name}_{ko}")
                    nc.sync.dma_start(t, xr[ko])
                    tiles.append(t)
                return tiles

            xq_t = load_x(xqT)
            nc.sync.dma_start(wk_sb, wkT[:])
            xk_t = load_x(xkT)
            nc.sync.dma_start(wv_sb, wvT[:])
            xv_t = load_x(xvT)
            if causal:
                cm_sb = consts.tile([P, 2, P], bf16)
                nc.sync.dma_start(cm_sb[:], cmask[:])
            nc.sync.dma_start(wo_sb, woT[:])

            # ---- memsets + HAM warm-up + ACT table preload -----------------
            warm = consts.tile([P, QSUP], bf16)
            nc.vector.memset(warm[:], 0.0)
            dummy = consts.tile([P, 1], f32)
            nc.scalar.activation(dummy[:], warm[:, 0:1], AF.Exp, scale=LN2)
            wp = stp.tile([P, 2, QSUP], f32, tag="ps2", name="warm_ps")
            for wi in range(10):
                nc.tensor.matmul(wp[:, 0, :], warm[:, 0:P], warm[:],
                                 start=(wi == 0), stop=(wi == 9))

            # qh/kh: per (m, superblock) FLAT tiles [128, 512]: partition p
            # holds dim p of the even head (p<64) / odd head (p>=64). The
            # row-tiled QK reads [0:64] / [64:128] directly and the bias-add
            # is a single full-partition tensor_scalar per superblock.
            qh_t = [[acts.tile([P, QSUP], bf16, name=f"qh_{m}_{ns}")
                     for ns in range(NSUP)] for m in range(2)]
            kh_t = [[acts.tile([P, QSUP], bf16, name=f"kh_{m}_{ns}")
                     for ns in range(NSUP)] for m in range(2)]
            vha = acts.tile([P, NKB, HPC, 2 * D], bf16)
            ctxT = acts.tile([P, 2, S], bf16)
            nc.vector.memset(vha[:, :, :, D:], 1.0)
            # warm-up liveness sink (writes exactly 1.0 into a ones column)
            nc.scalar.activation(vha[:, 0, 0, D:D + 1], wp[:, 0, 0:1],
                                 AF.Copy, bias=1.0, scale=0.0)

            # ---- projections ----------------------------------------------
            # chain layout per (proj, m): cA holds superblocks 0,1 (2 psum
            # banks), cB holds 2,3. Bias-adds are per-superblock so the
            # first QK only waits on the ns=0 K-chain.
            def proj_chains(m, xt, w_sb, tag=None, both_cpool=False):
                cA = cpool.tile([P, 2, QSUP], f32, tag="cp2",
                                name=f"pjA{m}_{tag}")
                if both_cpool:
                    # Km1 runs deferred inside the attention loop: it must
                    # not take an stp slot or it would poison the st
                    # ping-pong rotation with a late bias release.
                    cB = cpool.tile([P, 2, QSUP], f32, tag="cp2",
                                    name=f"pjB{m}_{tag}")
                else:
                    cB = stp.tile([P, 2, QSUP], f32, tag="ps2",
                                  name=f"pjB{m}_{tag}")
                chains = [(cA, 0), (cA, 1), (cB, 0), (cB, 1)]
                return cA, cB, chains

            def proj_ko(ko, m, xt, w_sb, chains):
                for ns in range(NSUP):
                    ct, half = chains[ns]
                    nc.tensor.matmul(
                        ct[:, half, :],
                        w_sb[:, ko, m * P:(m + 1) * P],
                        xt[ko][:, ns * QSUP:(ns + 1) * QSUP],
                        start=(ko == 0), stop=(ko == 7),
                    )

            def proj_bias(m, chains, bcol, dst):
                for ns in range(NSUP):
                    ct, half = chains[ns]
                    nc.vector.tensor_scalar_add(
                        dst[m][ns][:, :], ct[:, half, :],
                        bqk_sb[:, bcol + m:bcol + m + 1])

            # Q m=0 (xq-paced)
            _, _, qch0 = proj_chains(0, xq_t, wq_sb, tag="q")
            for ko in range(8):
                proj_ko(ko, 0, xq_t, wq_sb, qch0)
            proj_bias(0, qch0, 0, qh_t)

            # K m=0 interleaved with Q m=1: K MMs are xk-DMA-paced, Q m=1
            # fills the arrival gaps so the PE never idles into a HAM window.
            _, _, kch0 = proj_chains(0, xk_t, wk_sb, tag="k")
            _, _, qch1 = proj_chains(1, xq_t, wq_sb, tag="q")
            for ko in range(8):
                proj_ko(ko, 0, xk_t, wk_sb, kch0)
                proj_ko(ko, 1, xq_t, wq_sb, qch1)
            # K m0 bias FIRST on the DVE queue: it gates QK(0)/exp(0);
            # Qm1 bias is only needed at the m=1 units (iter 40).
            proj_bias(0, kch0, 2, kh_t)
            proj_bias(1, qch1, 0, qh_t)

            # ---- attention (ACT-bound; deferred PE work fills the slack) ---
            # m=0 descending (big at-buffered unit first), m=1 ASCENDING so
            # the stream ends on the big (3,1) unit whose AVs/outproj(2)
            # overlap its own exps; only outproj(3) + one norm trail.
            units = [(qs, 0) for qs in (3, 2, 1, 0)] + \
                    [(qs, 1) for qs in (3, 2, 1, 0)]
            iters = []
            for u, (qs, m) in enumerate(units):
                nkb = 4 * qs + 4 if causal else NKB
                for kb in range(nkb):
                    iters.append((u, qs, m, kb, nkb))
            n_it = len(iters)

            st_of = {}
            at_of = {}
            cps_of = {}

            def emit_qk(i):
                u, qs, m, kb, nkb = iters[i]
                r = kb - 4 * qs
                qlo = r * P if (causal and r >= 0) else 0
                st = stp.tile([P, 2, QSUP], f32, tag="ps2",
                              name=f"st_{u}_{kb}")
                ksb, kof = kb // 4, (kb % 4) * P
                for h2 in range(2):
                    pl = slice(h2 * D, h2 * D + D)
                    nc.tensor.matmul(
                        st[:, h2, qlo:],
                        kh_t[m][ksb][pl, kof:kof + P],
                        qh_t[m][qs][pl, qlo:],
                        start=True, stop=True,
                    )
                st_of[i] = (st, qlo)

            def emit_exp_mask(i):
                u, qs, m, kb, nkb = iters[i]
                st, qlo = st_of.pop(i)
                at = attn.tile([P, 2, QSUP], bf16, tag="at",
                               name=f"at_{u}_{kb}")
                nc.scalar.activation(at[:, :, qlo:], st[:, :, qlo:],
                                     AF.Exp, scale=LN2)
                if causal and kb - 4 * qs >= 0:
                    nc.vector.tensor_mul(
                        at[:, :, qlo:qlo + P], at[:, :, qlo:qlo + P], cm_sb,
                    )
                at_of[i] = (at, qlo)

            # deferred queue: (fn, cost_ns, min_iter)
            deferred = []

            def emit_av(i):
                u, qs, m, kb, nkb = iters[i]
                at, qlo = at_of.pop(i)
                if kb == 0:
                    cps_of[u] = cpool.tile([P, 2, QSUP], f32, tag="cp2",
                                           name=f"cps_{u}")
                cps = cps_of[u]
                for h2 in range(2):
                    h = 2 * m + h2
                    nc.tensor.matmul(
                        cps[:, h2, qlo:],
                        vha[:, kb, h, :],
                        at[:, h2, qlo:],
                        start=(kb == 0), stop=(kb == nkb - 1),
                    )

            def emit_norm(u):
                qs, m = units[u]
                cps = cps_of.pop(u)
                for h2 in range(2):
                    sums = norm.tile([D, QSUP], f32, tag="sums",
                                     name=f"sums_{u}_{h2}")
                    nc.vector.tensor_copy(out=sums, in_=cps[D:, h2, :])
                    rec = norm.tile([D, QSUP], f32, tag="rec",
                                    name=f"rec_{u}_{h2}")
                    nc.vector.reciprocal_approx_fast(out=rec, in_=sums)
                    nc.vector.tensor_mul(
                        ctxT[h2 * D:(h2 + 1) * D, m,
                             qs * QSUP:(qs + 1) * QSUP],
                        cps[0:D, h2, :], rec)

            pso_state = {}

            def emit_chunk(qs, sb, j):
                # qs==0 (the tail outproj): two psos ping-pong + ACT
                # copies (ScalarE is idle after the last exp)
                if qs == 0:
                    key = "a" if (j & 1) == 0 else "b"
                    if key not in pso_state:
                        pso_state[key] = cpool.tile(
                            [P, 2, QSUP], f32, tag="cp2", name=f"pso0{key}")
                    pso = pso_state[key]
                else:
                    if qs not in pso_state:
                        pso_state[qs] = cpool.tile(
                            [P, 2, QSUP], f32, tag="cp2", name=f"pso_{qs}")
                    pso = pso_state[qs]
                for km in range(2):
                    for n2 in range(2):
                        nc.tensor.matmul(
                            pso[:, n2, :],
                            ctxT[:, km, sb * P:(sb + 1) * P],
                            wo_sb[:, km, n2 * QSUP:(n2 + 1) * QSUP],
                            start=(km == 0), stop=(km == 1),
                        )
                ot = osb.tile([P, 2, QSUP], bf16, tag="ot", name=f"ot_{sb}")
                if qs == 0:
                    nc.scalar.activation(
                        ot.rearrange("p a b -> p (a b)"),
                        pso.rearrange("p a b -> p (a b)"),
                        AF.Copy,
                    )
                else:
                    nc.vector.tensor_copy(
                        out=ot.rearrange("p a b -> p (a b)"),
                        in_=pso.rearrange("p a b -> p (a b)"),
                    )
                nc.sync.dma_start(
                    out[sb * P:(sb + 1) * P, :],
                    ot.rearrange("p a b -> p (a b)"),
                )

            # --- pre-seed the deferred queue -------------------------------
            # Q m=1 then K m=1 projections (per-ko pop items + bias items),
            # with the V chains interleaved 2:1 between the K m=1 items so
            # both finish in time (kh[1] needed at iter 40, vha from the
            # first popped AV).
            pch_box = {}

            def pm1_ko(xt, w_sb, key, ko):
                def fn():
                    if key not in pch_box:
                        pch_box[key] = proj_chains(1, xt, w_sb, tag=key,
                                                   both_cpool=True)
                    _, _, ch = pch_box[key]
                    proj_ko(ko, 1, xt, w_sb, ch)
                return fn

            def pm1_bias(key, bcol, dst):
                def fn():
                    _, _, ch = pch_box[key]
                    proj_bias(1, ch, bcol, dst)
                return fn

            def v_chain(sb):
                def fn():
                    ps = cpool.tile([P, 2, QSUP], f32, tag="cp2",
                                    name=f"vp_{sb}")
                    for ko in range(8):
                        nc.tensor.matmul(
                            ps[:, 0, 0:DC],
                            xv_t[ko][:, sb * P:(sb + 1) * P],
                            wv_sb[:, ko, :],
                            start=(ko == 0), stop=(ko == 7),
                        )
                    nc.vector.tensor_copy(
                        vha[:, sb, :, 0:D],
                        ps[:, 0, 0:DC].rearrange("p (h d) -> p h d", h=HPC),
                    )
                return fn

            # FIFO: Km1 (needed by iter 40), then V chains (gated on the
            # xv DMA tail), then the in-loop AV/norm/outproj appends.
            for ko in range(8):
                deferred.append((pm1_ko(xk_t, wk_sb, "k1", ko), 900,
                                 1 + ko // 2))
            deferred.append((pm1_bias("k1", 2, kh_t), 100, 5))
            for sb in range(NKB):
                deferred.append((v_chain(sb), 950, 5 + sb // 2))
            # everything above MUST be emitted before the first m=1 QK is
            # emitted (iteration FORCE_BY), else that QK reads kh_t[1]
            # before its writes exist in the program -> no dep -> garbage
            n_seed = len(deferred)
            first_m1 = next(ix for ix, it in enumerate(iters) if it[2] == 1)
            FORCE_BY = first_m1 - 7

            # --- main attention loop ---------------------------------------
            credit = 0.0
            n_popped = 0
            emit_qk(0)
            for i in range(n_it):
                u, qs, m, kb, nkb = iters[i]
                r = kb - 4 * qs
                qlo = r * P if (causal and r >= 0) else 0
                cols = QSUP - qlo
                if i >= FORCE_BY:
                    while n_popped < n_seed:
                        fn, cost, mi = deferred.pop(0)
                        fn()
                        n_popped += 1
                emit_exp_mask(i)
                if i + 1 < n_it:
                    emit_qk(i + 1)

                # enqueue this iteration's AV (runs once popped)
                def av_fn(ii=i):
                    return emit_av(ii)
                deferred.append((av_fn, 2 * cols / 2.4, i + 1))
                if kb == nkb - 1:
                    def norm_fn(uu=u):
                        return emit_norm(uu)
                    deferred.append((norm_fn, 120, i + 1))
                    if m == 1:
                        for jj, sb in enumerate(range(4 * qs, 4 * qs + 4)):
                            def op_fn(q_=qs, s_=sb, j_=jj):
                                return emit_chunk(q_, s_, j_)
                            deferred.append((op_fn, 950, i + 2))

                # pops run BEFORE the 2-ahead QK: its st-slot wait (on
                # exp(i) completing) must not block them
                credit += (2 * cols * 0.8 + 295) - (cols / 2.4 + 100)
                credit = min(credit, 2600.0)
                while deferred:
                    fn, cost, mi = deferred[0]
                    if mi > i or cost > credit:
                        break
                    deferred.pop(0)
                    fn()
                    credit -= cost
                    n_popped += 1

            while deferred:
                fn, cost, mi = deferred.pop(0)
                fn()

    nc.finalize()
    return nc


def _get_nc(causal: bool):
    key = ("nc", causal)
    if key not in _CACHE:
        _CACHE[key] = _build_nc(causal)
    return _CACHE[key]


def _bf(a):
    return np.ascontiguousarray(a, dtype=np.float32).astype(BF16)


def _wperm(wT, nko):
    """[nko*128, M] -> [128, nko, M] so each SBUF partition's data is one
    contiguous run in DRAM (single DMA descriptor per partition)."""
    wT = np.asarray(wT, np.float32)
    m = wT.shape[1]
    return np.ascontiguousarray(
        wT.reshape(nko, P, m).transpose(1, 0, 2)).astype(BF16)


def kernel(q, k, v, mask, Wq, bq, Wk, bk, Wv, bv, Wo, bo):
    q = np.asarray(q, np.float32)
    k = np.asarray(k, np.float32)
    v = np.asarray(v, np.float32)
    mask = np.asarray(mask)
    Wq, bq = np.asarray(Wq, np.float32), np.asarray(bq, np.float32)
    Wk, bk = np.asarray(Wk, np.float32), np.asarray(bk, np.float32)
    Wv, bv = np.asarray(Wv, np.float32), np.asarray(bv, np.float32)
    Wo, bo = np.asarray(Wo, np.float32), np.asarray(bo, np.float32)

    m2 = mask.reshape(S, S) != 0
    if m2.all():
        causal = False
    else:
        tri = np.tril(np.ones((S, S), bool))
        assert (m2 == tri).all(), "only causal or all-ones masks supported"
        causal = True

    nc = _get_nc(causal)

    cm1 = np.asarray(
        np.arange(P)[:, None] <= np.arange(P)[None, :], np.float32
    ).astype(BF16)  # [k, q] keep-region of the diagonal 128-band
    cm = np.ascontiguousarray(
        np.broadcast_to(cm1[:, None, :], (P, 2, P))).astype(BF16)

    xT = {}
    for b in range(B):
        xT[("q", b)] = _bf(q[b].T)
        xT[("k", b)] = _bf(k[b].T)
        xT[("v", b)] = _bf(v[b].T)

    # log2(e)/sqrt(D) folded into Wq/bq: scores come out in the log2 domain
    qscale = LOG2E / SCALE
    in_maps = []
    for c in range(NCORES):
        b = c // 4
        rows = slice((c % 4) * DC, (c % 4) * DC + DC)
        bq_s = (bq[rows] * qscale).reshape(2, P).T
        bk_s = bk[rows].reshape(2, P).T
        in_maps.append({
            "xqT": xT[("q", b)],
            "xkT": xT[("k", b)],
            "xvT": xT[("v", b)],
            "wqT": _wperm(Wq[rows].T * qscale, 8),
            "wkT": _wperm(Wk[rows].T, 8),
            "wvT": _wperm(Wv[rows].T, 8),
            "woT": _wperm(Wo[:, rows].T, 2),
            "bqk": np.ascontiguousarray(
                np.concatenate([bq_s, bk_s], axis=1), np.float32),
            "cmask": cm,
        })

    res = run_bass_kernel_spmd(nc, in_maps, core_ids=list(range(NCORES)))
    LAST["exec_time_ns"] = res.exec_time_ns
    LAST["results"] = res

    host_bias = (bo + bv @ Wo.T).astype(np.float32)
    out = np.zeros((B, S, E), np.float32)
    for c in range(NCORES):
        out[c // 4] += res.results[c]["out"].astype(np.float32)
    out += host_bias
    return out


# revision 29
# speedup vs baseline: 1.0012x; 1.0012x over previous
"""Multi-head causal self-attention on 8 Trainium2 NeuronCores.

Problem: B=2, S=2048, E=1024, H=16 heads (D=64), causal mask, f32 I/O.

Sharding: (batch x head-group) -> 8 cores, as in the r* baseline (column-
parallel QKV, local attention, row-parallel partial output projection,
host-side partial sum + bias fold).

v5 restructure (exp-bound schedule):
  - Trace analysis showed attention is ScalarE(exp)-bound (~1.15us/iter vs
    PE ~0.65us) and the exp stream only started at 64us because Q,V,K
    projections fully serialize ahead of it on the in-order PE queue.
  - New DMA order xq, xk, xv: attention pair-0 exp stream starts right
    after the K m=0 projection (~34us), bounded by the 9MB q+k input DMA.
  - All non-critical PE work (K m=1 projection, V-projection chains, AV
    matmuls, output projection) is deferred into a budgeted pop queue that
    fills the ~0.5us/iter PE slack of the ACT-bound attention loop.
    Deferred AVs buffer their attn tiles in SBUF (at pool bufs=26).
  - QK is emitted as two concurrent row-tiled K=64 matmuls (even head in
    partitions 0:64, odd in 64:128 - layout already provided it); halves
    QK PE time and removes the need to zero-pad qh/kh.
  - qh/kh live as per-(m, superblock) tiles so the first QK only waits on
    the first K-chain bias-add, not the whole projection.
  - pop items carry (cost, min_iter): min_iter keeps a pop whose deps lie
    in the future out of the PE queue (head-of-line), cost keeps the
    per-iteration emitted PE work under the exp period so QK(i+1) never
    drifts behind and ScalarE never starves.
"""

import os
import sys

for _p in ("/opt/trn_rl_repo",):
    if _p not in sys.path and os.path.isdir(_p):
        sys.path.insert(0, _p)

import numpy as np
import ml_dtypes

import concourse.bacc as bacc
from concourse import mybir
from concourse.tile import TileContext
from concourse.bass_utils import run_bass_kernel_spmd

BF16 = ml_dtypes.bfloat16
P = 128
B, S, E, H, D = 2, 2048, 1024, 16, 64
HPC = 4            # heads per core
DC = HPC * D       # 256 output dims per core per projection
NCORES = 8
QSUP = 512         # q-superblock (matmul free dim)
NSUP = S // QSUP   # 4
NKB = S // P       # 16 k-blocks
SCALE = float(np.sqrt(D))
LOG2E = float(np.log2(np.e))
LN2 = float(np.log(2.0))

AF = mybir.ActivationFunctionType
f32 = mybir.dt.float32
bf16 = mybir.dt.bfloat16

_CACHE = {}
LAST = {}


def _install_axon_profile_shim():
    """Provide antenv.axon_hooks (absent in this image) so
    run_bass_kernel_spmd(trace=True) can NTFF-profile via libaxon_pjrt.so."""
    try:
        import antenv.axon_hooks  # noqa: F401
        return
    except ImportError:
        pass
    import contextlib
    import ctypes
    import types

    import antenv

    state = {"hook": None, "tried": False}

    def _build_hook():
        so_path = "/opt/axon/libaxon_pjrt.so"
        if not os.path.exists(so_path):
            return None
        lib = ctypes.CDLL(so_path)
        if not hasattr(lib, "axon_start_nrt_profile"):
            return None
        lib.axon_start_nrt_profile.argtypes = [
            ctypes.POINTER(ctypes.c_int64),
            ctypes.c_size_t,
        ]
        lib.axon_start_nrt_profile.restype = ctypes.c_int64
        lib.axon_stop_nrt_profile.argtypes = [ctypes.c_char_p]
        lib.axon_stop_nrt_profile.restype = ctypes.c_int64

        @contextlib.contextmanager
        def _hook(output_dir, device_ids):
            import jax

            jax.devices()
            if device_ids:
                ids = (ctypes.c_int64 * len(device_ids))(*device_ids)
                rc = lib.axon_start_nrt_profile(ids, len(device_ids))
            else:
                rc = lib.axon_start_nrt_profile(None, 0)
            if rc != 0:
                raise RuntimeError(f"axon_start_nrt_profile rc={rc}")
            try:
                yield
            finally:
                n = lib.axon_stop_nrt_profile(str(output_dir).encode())
                if n < 0:
                    raise RuntimeError(f"axon_stop_nrt_profile rc={n}")
                print(f"profile: {n} file(s) written to {output_dir}")

        return _hook

    mod = types.ModuleType("antenv.axon_hooks")

    def set_axon_ntff_profile_hook(h):
        state["hook"] = h
        state["tried"] = True

    def get_axon_ntff_profile_hook():
        if not state["tried"]:
            state["hook"] = _build_hook()
            state["tried"] = True
        return state["hook"]

    mod.set_axon_ntff_profile_hook = set_axon_ntff_profile_hook
    mod.get_axon_ntff_profile_hook = get_axon_ntff_profile_hook
    sys.modules["antenv.axon_hooks"] = mod
    antenv.axon_hooks = mod


_install_axon_profile_shim()


def _build_nc(causal: bool):
    nc = bacc.Bacc(None, target_bir_lowering=False)

    xqT = nc.dram_tensor("xqT", [E, S], bf16, kind="ExternalInput")
    xkT = nc.dram_tensor("xkT", [E, S], bf16, kind="ExternalInput")
    xvT = nc.dram_tensor("xvT", [E, S], bf16, kind="ExternalInput")
    wqT = nc.dram_tensor("wqT", [P, 8, DC], bf16, kind="ExternalInput")
    wkT = nc.dram_tensor("wkT", [P, 8, DC], bf16, kind="ExternalInput")
    wvT = nc.dram_tensor("wvT", [P, 8, DC], bf16, kind="ExternalInput")
    woT = nc.dram_tensor("woT", [P, 2, E], bf16, kind="ExternalInput")
    bqk = nc.dram_tensor("bqk", [P, 4], f32, kind="ExternalInput")
    cmask = nc.dram_tensor("cmask", [P, 2, P], bf16, kind="ExternalInput")
    out = nc.dram_tensor("out", [S, E], bf16, kind="ExternalOutput")

    with TileContext(nc) as tc:
        with (
            tc.tile_pool(name="consts", bufs=1) as consts,
            tc.tile_pool(name="xin", bufs=16) as xin,
            tc.tile_pool(name="acts", bufs=1) as acts,
            tc.tile_pool(name="attn", bufs=26) as attn,
            tc.tile_pool(name="norm", bufs=4) as norm,
            tc.tile_pool(name="osb", bufs=3) as osb,
            tc.tile_pool(name="stp", bufs=2, space="PSUM") as stp,
            tc.tile_pool(name="cpool", bufs=2, space="PSUM") as cpool,
        ):
            # ---- input DMAs: q, k first (critical path), v last ------------
            wq_sb = consts.tile([P, 8, DC], bf16)
            wk_sb = consts.tile([P, 8, DC], bf16)
            wv_sb = consts.tile([P, 8, DC], bf16)
            wo_sb = consts.tile([P, 2, E], bf16)
            bqk_sb = consts.tile([P, 4], f32)
            nc.sync.dma_start(wq_sb, wqT[:])
            nc.sync.dma_start(bqk_sb[:], bqk[:])

            def load_x(xT):
                xr = xT.rearrange("(ko p) s -> ko p s", p=P)
                tiles = []
                for ko in range(8):
                    t = xin.tile([P, S], bf16, tag="xin", name=f"x_{xT.name}_{ko}")
                    nc.sync.dma_start(t, xr[ko])
                    tiles.append(t)
                return tiles

            xq_t = load_x(xqT)
            nc.sync.dma_start(wk_sb, wkT[:])
            xk_t = load_x(xkT)
            nc.sync.dma_start(wv_sb, wvT[:])
            xv_t = load_x(xvT)
            if causal:
                cm_sb = consts.tile([P, 2, P], bf16)
                nc.sync.dma_start(cm_sb[:], cmask[:])
            nc.sync.dma_start(wo_sb, woT[:])

            # ---- memsets + HAM warm-up + ACT table preload -----------------
            warm = consts.tile([P, QSUP], bf16)
            nc.vector.memset(warm[:], 0.0)
            dummy = consts.tile([P, 1], f32)
            nc.scalar.activation(dummy[:], warm[:, 0:1], AF.Exp, scale=LN2)
            wp = stp.tile([P, 2, QSUP], f32, tag="ps2", name="warm_ps")
            for wi in range(10):
                nc.tensor.matmul(wp[:, 0, :], warm[:, 0:P], warm[:],
                                 start=(wi == 0), stop=(wi == 9))

            # qh/kh: per (m, superblock) FLAT tiles [128, 512]: partition p
            # holds dim p of the even head (p<64) / odd head (p>=64). The
            # row-tiled QK reads [0:64] / [64:128] directly and the bias-add
            # is a single full-partition tensor_scalar per superblock.
            qh_t = [[acts.tile([P, QSUP], bf16, name=f"qh_{m}_{ns}")
                     for ns in range(NSUP)] for m in range(2)]
            kh_t = [[acts.tile([P, QSUP], bf16, name=f"kh_{m}_{ns}")
                     for ns in range(NSUP)] for m in range(2)]
            vha = acts.tile([P, NKB, HPC, 2 * D], bf16)
            ctxT = acts.tile([P, 2, S], bf16)
            nc.vector.memset(vha[:, :, :, D:], 1.0)
            # warm-up liveness sink (writes exactly 1.0 into a ones column)
            nc.scalar.activation(vha[:, 0, 0, D:D + 1], wp[:, 0, 0:1],
                                 AF.Copy, bias=1.0, scale=0.0)

            # ---- projections ----------------------------------------------
            # chain layout per (proj, m): cA holds superblocks 0,1 (2 psum
            # banks), cB holds 2,3. Bias-adds are per-superblock so the
            # first QK only waits on the ns=0 K-chain.
            def proj_chains(m, xt, w_sb, tag=None, both_cpool=False):
                cA = cpool.tile([P, 2, QSUP], f32, tag="cp2",
                                name=f"pjA{m}_{tag}")
                if both_cpool:
                    # Km1 runs deferred inside the attention loop: it must
                    # not take an stp slot or it would poison the st
                    # ping-pong rotation with a late bias release.
                    cB = cpool.tile([P, 2, QSUP], f32, tag="cp2",
                                    name=f"pjB{m}_{tag}")
                else:
                    cB = stp.tile([P, 2, QSUP], f32, tag="ps2",
                                  name=f"pjB{m}_{tag}")
                chains = [(cA, 0), (cA, 1), (cB, 0), (cB, 1)]
                return cA, cB, chains

            def proj_ko(ko, m, xt, w_sb, chains):
                for ns in range(NSUP):
                    ct, half = chains[ns]
                    nc.tensor.matmul(
                        ct[:, half, :],
                        w_sb[:, ko, m * P:(m + 1) * P],
                        xt[ko][:, ns * QSUP:(ns + 1) * QSUP],
                        start=(ko == 0), stop=(ko == 7),
                    )

            def proj_bias(m, chains, bcol, dst):
                for ns in range(NSUP):
                    ct, half = chains[ns]
                    nc.vector.tensor_scalar_add(
                        dst[m][ns][:, :], ct[:, half, :],
                        bqk_sb[:, bcol + m:bcol + m + 1])

            # Q m=0 (xq-paced)
            _, _, qch0 = proj_chains(0, xq_t, wq_sb, tag="q")
            for ko in range(8):
                proj_ko(ko, 0, xq_t, wq_sb, qch0)
            proj_bias(0, qch0, 0, qh_t)

            # K m=0 interleaved with Q m=1: K MMs are xk-DMA-paced, Q m=1
            # fills the arrival gaps so the PE never idles into a HAM window.
            _, _, kch0 = proj_chains(0, xk_t, wk_sb, tag="k")
            _, _, qch1 = proj_chains(1, xq_t, wq_sb, tag="q")
            for ko in range(8):
                proj_ko(ko, 0, xk_t, wk_sb, kch0)
                proj_ko(ko, 1, xq_t, wq_sb, qch1)
            # K m0 bias FIRST on the DVE queue: it gates QK(0)/exp(0);
            # Qm1 bias is only needed at the m=1 units (iter 40).
            proj_bias(0, kch0, 2, kh_t)
            proj_bias(1, qch1, 0, qh_t)

            # ---- attention (ACT-bound; deferred PE work fills the slack) ---
            # m=0 descending (big at-buffered unit first), m=1 ASCENDING so
            # the stream ends on the big (3,1) unit whose AVs/outproj(2)
            # overlap its own exps; only outproj(3) + one norm trail.
            units = [(qs, 0) for qs in (3, 2, 1, 0)] + \
                    [(qs, 1) for qs in (3, 2, 1, 0)]
            iters = []
            for u, (qs, m) in enumerate(units):
                nkb = 4 * qs + 4 if causal else NKB
                for kb in range(nkb):
                    iters.append((u, qs, m, kb, nkb))
            n_it = len(iters)

            st_of = {}
            at_of = {}
            cps_of = {}

            def emit_qk(i):
                u, qs, m, kb, nkb = iters[i]
                r = kb - 4 * qs
                qlo = r * P if (causal and r >= 0) else 0
                st = stp.tile([P, 2, QSUP], f32, tag="ps2",
                              name=f"st_{u}_{kb}")
                ksb, kof = kb // 4, (kb % 4) * P
                for h2 in range(2):
                    pl = slice(h2 * D, h2 * D + D)
                    nc.tensor.matmul(
                        st[:, h2, qlo:],
                        kh_t[m][ksb][pl, kof:kof + P],
                        qh_t[m][qs][pl, qlo:],
                        start=True, stop=True,
                    )
                st_of[i] = (st, qlo)

            def emit_exp_mask(i):
                u, qs, m, kb, nkb = iters[i]
                st, qlo = st_of.pop(i)
                at = attn.tile([P, 2, QSUP], bf16, tag="at",
                               name=f"at_{u}_{kb}")
                nc.scalar.activation(at[:, :, qlo:], st[:, :, qlo:],
                                     AF.Exp, scale=LN2)
                if causal and kb - 4 * qs >= 0:
                    nc.vector.tensor_mul(
                        at[:, :, qlo:qlo + P], at[:, :, qlo:qlo + P], cm_sb,
                    )
                at_of[i] = (at, qlo)

            # deferred queue: (fn, cost_ns, min_iter)
            deferred = []

            def emit_av(i):
                u, qs, m, kb, nkb = iters[i]
                at, qlo = at_of.pop(i)
                if kb == 0:
                    cps_of[u] = cpool.tile([P, 2, QSUP], f32, tag="cp2",
                                           name=f"cps_{u}")
                cps = cps_of[u]
                for h2 in range(2):
                    h = 2 * m + h2
                    nc.tensor.matmul(
                        cps[:, h2, qlo:],
                        vha[:, kb, h, :],
                        at[:, h2, qlo:],
                        start=(kb == 0), stop=(kb == nkb - 1),
                    )

            def emit_norm(u):
                qs, m = units[u]
                cps = cps_of.pop(u)
                for h2 in range(2):
                    sums = norm.tile([D, QSUP], f32, tag="sums",
                                     name=f"sums_{u}_{h2}")
                    nc.vector.tensor_copy(out=sums, in_=cps[D:, h2, :])
                    rec = norm.tile([D, QSUP], f32, tag="rec",
                                    name=f"rec_{u}_{h2}")
                    nc.vector.reciprocal_approx_fast(out=rec, in_=sums)
                    nc.vector.tensor_mul(
                        ctxT[h2 * D:(h2 + 1) * D, m,
                             qs * QSUP:(qs + 1) * QSUP],
                        cps[0:D, h2, :], rec)

            pso_state = {}

            def emit_chunk(qs, sb, j):
                # qs==0 (the tail outproj): two psos ping-pong + ACT
                # copies (ScalarE is idle after the last exp)
                if qs == 0:
                    key = "a" if (j & 1) == 0 else "b"
                    if key not in pso_state:
                        pso_state[key] = cpool.tile(
                            [P, 2, QSUP], f32, tag="cp2", name=f"pso0{key}")
                    pso = pso_state[key]
                else:
                    if qs not in pso_state:
                        pso_state[qs] = cpool.tile(
                            [P, 2, QSUP], f32, tag="cp2", name=f"pso_{qs}")
                    pso = pso_state[qs]
                for km in range(2):
                    for n2 in range(2):
                        nc.tensor.matmul(
                            pso[:, n2, :],
                            ctxT[:, km, sb * P:(sb + 1) * P],
                            wo_sb[:, km, n2 * QSUP:(n2 + 1) * QSUP],
                            start=(km == 0), stop=(km == 1),
                        )
                ot = osb.tile([P, 2, QSUP], bf16, tag="ot", name=f"ot_{sb}")
                if qs == 0:
                    nc.scalar.activation(
                        ot.rearrange("p a b -> p (a b)"),
                        pso.rearrange("p a b -> p (a b)"),
                        AF.Copy,
                    )
                else:
                    nc.vector.tensor_copy(
                        out=ot.rearrange("p a b -> p (a b)"),
                        in_=pso.rearrange("p a b -> p (a b)"),
                    )
                nc.sync.dma_start(
                    out[sb * P:(sb + 1) * P, :],
                    ot.rearrange("p a b -> p (a b)"),
                )

            # --- pre-seed the deferred queue -------------------------------
            # Q m=1 then K m=1 projections (per-ko pop items + bias items),
            # with the V chains interleaved 2:1 between the K m=1 items so
            # both finish in time (kh[1] needed at iter 40, vha from the
            # first popped AV).
            pch_box = {}

            def pm1_ko(xt, w_sb, key, ko):
                def fn():
                    if key not in pch_box:
                        pch_box[key] = proj_chains(1, xt, w_sb, tag=key,
                                                   both_cpool=True)
                    _, _, ch = pch_box[key]
                    proj_ko(ko, 1, xt, w_sb, ch)
                return fn

            def pm1_bias(key, bcol, dst):
                def fn():
                    _, _, ch = pch_box[key]
                    proj_bias(1, ch, bcol, dst)
                return fn

            def v_chain(sb):
                def fn():
                    ps = cpool.tile([P, 2, QSUP], f32, tag="cp2",
                                    name=f"vp_{sb}")
                    for ko in range(8):
                        nc.tensor.matmul(
                            ps[:, 0, 0:DC],
                            xv_t[ko][:, sb * P:(sb + 1) * P],
                            wv_sb[:, ko, :],
                            start=(ko == 0), stop=(ko == 7),
                        )
                    nc.vector.tensor_copy(
                        vha[:, sb, :, 0:D],
                        ps[:, 0, 0:DC].rearrange("p (h d) -> p h d", h=HPC),
                    )
                return fn

            # FIFO: Km1 (needed by iter 40), then V chains (gated on the
            # xv DMA tail), then the in-loop AV/norm/outproj appends.
            for ko in range(8):
                deferred.append((pm1_ko(xk_t, wk_sb, "k1", ko), 900,
                                 1 + ko // 2))
            deferred.append((pm1_bias("k1", 2, kh_t), 100, 5))
            for sb in range(NKB):
                deferred.append((v_chain(sb), 950, 5 + sb // 2))
            # everything above MUST be emitted before the first m=1 QK is
            # emitted (iteration FORCE_BY), else that QK reads kh_t[1]
            # before its writes exist in the program -> no dep -> garbage
            n_seed = len(deferred)
            first_m1 = next(ix for ix, it in enumerate(iters) if it[2] == 1)
            FORCE_BY = first_m1 - 7

            # --- main attention loop ---------------------------------------
            credit = 0.0
            n_popped = 0
            emit_qk(0)
            for i in range(n_it):
                u, qs, m, kb, nkb = iters[i]
                r = kb - 4 * qs
                qlo = r * P if (causal and r >= 0) else 0
                cols = QSUP - qlo
                if i >= FORCE_BY:
                    while n_popped < n_seed:
                        fn, cost, mi = deferred.pop(0)
                        fn()
                        n_popped += 1
                emit_exp_mask(i)
                if i + 1 < n_it:
                    emit_qk(i + 1)

                # enqueue this iteration's AV (runs once popped)
                def av_fn(ii=i):
                    return emit_av(ii)
                deferred.append((av_fn, 2 * cols / 2.4, i + 1))
                if kb == nkb - 1:
                    def norm_fn(uu=u):
                        return emit_norm(uu)
                    deferred.append((norm_fn, 120, i + 1))
                    if m == 1:
                        for jj, sb in enumerate(range(4 * qs, 4 * qs + 4)):
                            def op_fn(q_=qs, s_=sb, j_=jj):
                                return emit_chunk(q_, s_, j_)
                            deferred.append((op_fn, 950, i + 2))

                # pops run BEFORE the 2-ahead QK: its st-slot wait (on
                # exp(i) completing) must not block them
                credit += (2 * cols * 0.8 + 295) - (cols / 2.4 + 100)
                credit = min(credit, 2600.0)
                while deferred:
                    fn, cost, mi = deferred[0]
                    if mi > i or cost > credit:
                        break
                    deferred.pop(0)
                    fn()
                    credit -= cost
                    n_popped += 1

            while deferred:
                fn, cost, mi = deferred.pop(0)
                fn()

    nc.finalize()
    return nc


def _get_nc(causal: bool):
    key = ("nc", causal)
    if key not in _CACHE:
        _CACHE[key] = _build_nc(causal)
    return _CACHE[key]


def _bf(a):
    return np.ascontiguousarray(a, dtype=np.float32).astype(BF16)


def _wperm(wT, nko):
    """[nko*128, M] -> [128, nko, M] so each SBUF partition's data is one
    contiguous run in DRAM (single DMA descriptor per partition)."""
    wT = np.asarray(wT, np.float32)
    m = wT.shape[1]
    return np.ascontiguousarray(
        wT.reshape(nko, P, m).transpose(1, 0, 2)).astype(BF16)


def kernel(q, k, v, mask, Wq, bq, Wk, bk, Wv, bv, Wo, bo):
    q = np.asarray(q, np.float32)
    k = np.asarray(k, np.float32)
    v = np.asarray(v, np.float32)
    mask = np.asarray(mask)
    Wq, bq = np.asarray(Wq, np.float32), np.asarray(bq, np.float32)
    Wk, bk = np.asarray(Wk, np.float32), np.asarray(bk, np.float32)
    Wv, bv = np.asarray(Wv, np.float32), np.asarray(bv, np.float32)
    Wo, bo = np.asarray(Wo, np.float32), np.asarray(bo, np.float32)

    m2 = mask.reshape(S, S) != 0
    if m2.all():
        causal = False
    else:
        tri = np.tril(np.ones((S, S), bool))
        assert (m2 == tri).all(), "only causal or all-ones masks supported"
        causal = True

    nc = _get_nc(causal)

    cm1 = np.asarray(
        np.arange(P)[:, None] <= np.arange(P)[None, :], np.float32
    ).astype(BF16)  # [k, q] keep-region of the diagonal 128-band
    cm = np.ascontiguousarray(
        np.broadcast_to(cm1[:, None, :], (P, 2, P))).astype(BF16)

    xT = {}
    for b in range(B):
        xT[("q", b)] = _bf(q[b].T)
        xT[("k", b)] = _bf(k[b].T)
        xT[("v", b)] = _bf(v[b].T)

    # log2(e)/sqrt(D) folded into Wq/bq: scores come out in the log2 domain
    qscale = LOG2E / SCALE
    in_maps = []
    for c in range(NCORES):
        b = c // 4
        rows = slice((c % 4) * DC, (c % 4) * DC + DC)
        bq_s = (bq[rows] * qscale).reshape(2, P).T
        bk_s = bk[rows].reshape(2, P).T
        in_maps.append({
            "xqT": xT[("q", b)],
            "xkT": xT[("k", b)],
            "xvT": xT[("v", b)],
            "wqT": _wperm(Wq[rows].T * qscale, 8),
            "wkT": _wperm(Wk[rows].T, 8),
            "wvT": _wperm(Wv[rows].T, 8),
            "woT": _wperm(Wo[:, rows].T, 2),
            "bqk": np.ascontiguousarray(
                np.concatenate([bq_s, bk_s], axis=1), np.float32),
            "cmask": cm,
        })

    res = run_bass_kernel_spmd(nc, in_maps, core_ids=list(range(NCORES)))
    LAST["exec_time_ns"] = res.exec_time_ns
    LAST["results"] = res

    host_bias = (bo + bv @ Wo.T).astype(np.float32)
    out = np.zeros((B, S, E), np.float32)
    for c in range(NCORES):
        out[c // 4] += res.results[c]["out"].astype(np.float32)
    out += host_bias
    return out


# revision 30
# speedup vs baseline: 1.0349x; 1.0337x over previous
"""Multi-head causal self-attention on 8 Trainium2 NeuronCores.

Problem: B=2, S=2048, E=1024, H=16 heads (D=64), causal mask, f32 I/O.

Sharding: (batch x head-group) -> 8 cores, as in the r* baseline (column-
parallel QKV, local attention, row-parallel partial output projection,
host-side partial sum + bias fold).

v5 restructure (exp-bound schedule):
  - Trace analysis showed attention is ScalarE(exp)-bound (~1.15us/iter vs
    PE ~0.65us) and the exp stream only started at 64us because Q,V,K
    projections fully serialize ahead of it on the in-order PE queue.
  - New DMA order xq, xk, xv: attention pair-0 exp stream starts right
    after the K m=0 projection (~34us), bounded by the 9MB q+k input DMA.
  - All non-critical PE work (K m=1 projection, V-projection chains, AV
    matmuls, output projection) is deferred into a budgeted pop queue that
    fills the ~0.5us/iter PE slack of the ACT-bound attention loop.
    Deferred AVs buffer their attn tiles in SBUF (at pool bufs=26).
  - QK is emitted as two concurrent row-tiled K=64 matmuls (even head in
    partitions 0:64, odd in 64:128 - layout already provided it); halves
    QK PE time and removes the need to zero-pad qh/kh.
  - qh/kh live as per-(m, superblock) tiles so the first QK only waits on
    the first K-chain bias-add, not the whole projection.
  - pop items carry (cost, min_iter): min_iter keeps a pop whose deps lie
    in the future out of the PE queue (head-of-line), cost keeps the
    per-iteration emitted PE work under the exp period so QK(i+1) never
    drifts behind and ScalarE never starves.
"""

import os
import sys

for _p in ("/opt/trn_rl_repo",):
    if _p not in sys.path and os.path.isdir(_p):
        sys.path.insert(0, _p)

import numpy as np
import ml_dtypes

import concourse.bacc as bacc
from concourse import mybir
from concourse.tile import TileContext
from concourse.bass_utils import run_bass_kernel_spmd

BF16 = ml_dtypes.bfloat16
P = 128
B, S, E, H, D = 2, 2048, 1024, 16, 64
HPC = 4            # heads per core
DC = HPC * D       # 256 output dims per core per projection
NCORES = 8
QSUP = 512         # q-superblock (matmul free dim)
NSUP = S // QSUP   # 4
NKB = S // P       # 16 k-blocks
SCALE = float(np.sqrt(D))
LOG2E = float(np.log2(np.e))
LN2 = float(np.log(2.0))

AF = mybir.ActivationFunctionType
f32 = mybir.dt.float32
bf16 = mybir.dt.bfloat16

_CACHE = {}
LAST = {}


def _install_axon_profile_shim():
    """Provide antenv.axon_hooks (absent in this image) so
    run_bass_kernel_spmd(trace=True) can NTFF-profile via libaxon_pjrt.so."""
    try:
        import antenv.axon_hooks  # noqa: F401
        return
    except ImportError:
        pass
    import contextlib
    import ctypes
    import types

    import antenv

    state = {"hook": None, "tried": False}

    def _build_hook():
        so_path = "/opt/axon/libaxon_pjrt.so"
        if not os.path.exists(so_path):
            return None
        lib = ctypes.CDLL(so_path)
        if not hasattr(lib, "axon_start_nrt_profile"):
            return None
        lib.axon_start_nrt_profile.argtypes = [
            ctypes.POINTER(ctypes.c_int64),
            ctypes.c_size_t,
        ]
        lib.axon_start_nrt_profile.restype = ctypes.c_int64
        lib.axon_stop_nrt_profile.argtypes = [ctypes.c_char_p]
        lib.axon_stop_nrt_profile.restype = ctypes.c_int64

        @contextlib.contextmanager
        def _hook(output_dir, device_ids):
            import jax

            jax.devices()
            if device_ids:
                ids = (ctypes.c_int64 * len(device_ids))(*device_ids)
                rc = lib.axon_start_nrt_profile(ids, len(device_ids))
            else:
                rc = lib.axon_start_nrt_profile(None, 0)
            if rc != 0:
                raise RuntimeError(f"axon_start_nrt_profile rc={rc}")
            try:
                yield
            finally:
                n = lib.axon_stop_nrt_profile(str(output_dir).encode())
                if n < 0:
                    raise RuntimeError(f"axon_stop_nrt_profile rc={n}")
                print(f"profile: {n} file(s) written to {output_dir}")

        return _hook

    mod = types.ModuleType("antenv.axon_hooks")

    def set_axon_ntff_profile_hook(h):
        state["hook"] = h
        state["tried"] = True

    def get_axon_ntff_profile_hook():
        if not state["tried"]:
            state["hook"] = _build_hook()
            state["tried"] = True
        return state["hook"]

    mod.set_axon_ntff_profile_hook = set_axon_ntff_profile_hook
    mod.get_axon_ntff_profile_hook = get_axon_ntff_profile_hook
    sys.modules["antenv.axon_hooks"] = mod
    antenv.axon_hooks = mod


_install_axon_profile_shim()


def _build_nc(causal: bool):
    nc = bacc.Bacc(None, target_bir_lowering=False)

    xqT = nc.dram_tensor("xqT", [E, S], bf16, kind="ExternalInput")
    xkT = nc.dram_tensor("xkT", [E, S], bf16, kind="ExternalInput")
    xvT = nc.dram_tensor("xvT", [E, S], bf16, kind="ExternalInput")
    wqT = nc.dram_tensor("wqT", [P, 8, DC], bf16, kind="ExternalInput")
    wkT = nc.dram_tensor("wkT", [P, 8, DC], bf16, kind="ExternalInput")
    wvT = nc.dram_tensor("wvT", [P, 8, DC], bf16, kind="ExternalInput")
    woT = nc.dram_tensor("woT", [P, 2, E], bf16, kind="ExternalInput")
    bqk = nc.dram_tensor("bqk", [P, 4], f32, kind="ExternalInput")
    cmask = nc.dram_tensor("cmask", [P, 2, P], bf16, kind="ExternalInput")
    out = nc.dram_tensor("out", [S, E], bf16, kind="ExternalOutput")

    with TileContext(nc) as tc:
        with (
            tc.tile_pool(name="consts", bufs=1) as consts,
            tc.tile_pool(name="xin", bufs=16) as xin,
            tc.tile_pool(name="acts", bufs=1) as acts,
            tc.tile_pool(name="attn", bufs=26) as attn,
            tc.tile_pool(name="norm", bufs=4) as norm,
            tc.tile_pool(name="osb", bufs=3) as osb,
            tc.tile_pool(name="stp", bufs=2, space="PSUM") as stp,
            tc.tile_pool(name="cpool", bufs=2, space="PSUM") as cpool,
        ):
            # ---- input DMAs: q, k first (critical path), v last ------------
            wq_sb = consts.tile([P, 8, DC], bf16)
            wk_sb = consts.tile([P, 8, DC], bf16)
            wv_sb = consts.tile([P, 8, DC], bf16)
            wo_sb = consts.tile([P, 2, E], bf16)
            bqk_sb = consts.tile([P, 4], f32)
            nc.sync.dma_start(wq_sb, wqT[:])
            nc.sync.dma_start(bqk_sb[:], bqk[:])

            def load_x(xT):
                xr = xT.rearrange("(ko p) s -> ko p s", p=P)
                tiles = []
                for ko in range(8):
                    t = xin.tile([P, S], bf16, tag="xin", name=f"x_{xT.name}_{ko}")
                    nc.sync.dma_start(t, xr[ko])
                    tiles.append(t)
                return tiles

            xq_t = load_x(xqT)
            nc.sync.dma_start(wk_sb, wkT[:])
            xk_t = load_x(xkT)
            nc.sync.dma_start(wv_sb, wvT[:])
            xv_t = load_x(xvT)
            if causal:
                cm_sb = consts.tile([P, 2, P], bf16)
                nc.sync.dma_start(cm_sb[:], cmask[:])
            nc.sync.dma_start(wo_sb, woT[:])

            # ---- memsets + HAM warm-up + ACT table preload -----------------
            warm = consts.tile([P, QSUP], bf16)
            nc.vector.memset(warm[:], 0.0)
            dummy = consts.tile([P, 1], f32)
            nc.scalar.activation(dummy[:], warm[:, 0:1], AF.Exp, scale=LN2)
            wp = stp.tile([P, 2, QSUP], f32, tag="ps2", name="warm_ps")
            for wi in range(10):
                nc.tensor.matmul(wp[:, 0, :], warm[:, 0:P], warm[:],
                                 start=(wi == 0), stop=(wi == 9))

            # qh/kh: per (m, superblock) FLAT tiles [128, 512]: partition p
            # holds dim p of the even head (p<64) / odd head (p>=64). The
            # row-tiled QK reads [0:64] / [64:128] directly and the bias-add
            # is a single full-partition tensor_scalar per superblock.
            qh_t = [[acts.tile([P, QSUP], bf16, name=f"qh_{m}_{ns}")
                     for ns in range(NSUP)] for m in range(2)]
            kh_t = [[acts.tile([P, QSUP], bf16, name=f"kh_{m}_{ns}")
                     for ns in range(NSUP)] for m in range(2)]
            vha = acts.tile([P, NKB, HPC, 2 * D], bf16)
            ctxT = acts.tile([P, 2, S], bf16)
            nc.vector.memset(vha[:, :, :, D:], 1.0)
            # warm-up liveness sink (writes exactly 1.0 into a ones column)
            nc.scalar.activation(vha[:, 0, 0, D:D + 1], wp[:, 0, 0:1],
                                 AF.Copy, bias=1.0, scale=0.0)

            # ---- projections ----------------------------------------------
            # chain layout per (proj, m): cA holds superblocks 0,1 (2 psum
            # banks), cB holds 2,3. Bias-adds are per-superblock so the
            # first QK only waits on the ns=0 K-chain.
            def proj_chains(m, xt, w_sb, tag=None, both_cpool=False):
                cA = cpool.tile([P, 2, QSUP], f32, tag="cp2",
                                name=f"pjA{m}_{tag}")
                if both_cpool:
                    # Km1 runs deferred inside the attention loop: it must
                    # not take an stp slot or it would poison the st
                    # ping-pong rotation with a late bias release.
                    cB = cpool.tile([P, 2, QSUP], f32, tag="cp2",
                                    name=f"pjB{m}_{tag}")
                else:
                    cB = stp.tile([P, 2, QSUP], f32, tag="ps2",
                                  name=f"pjB{m}_{tag}")
                chains = [(cA, 0), (cA, 1), (cB, 0), (cB, 1)]
                return cA, cB, chains

            def proj_ko(ko, m, xt, w_sb, chains):
                for ns in range(NSUP):
                    ct, half = chains[ns]
                    nc.tensor.matmul(
                        ct[:, half, :],
                        w_sb[:, ko, m * P:(m + 1) * P],
                        xt[ko][:, ns * QSUP:(ns + 1) * QSUP],
                        start=(ko == 0), stop=(ko == 7),
                    )

            def proj_bias(m, chains, bcol, dst):
                for ns in range(NSUP):
                    ct, half = chains[ns]
                    nc.vector.tensor_scalar_add(
                        dst[m][ns][:, :], ct[:, half, :],
                        bqk_sb[:, bcol + m:bcol + m + 1])

            # Q m=0 (xq-paced)
            _, _, qch0 = proj_chains(0, xq_t, wq_sb, tag="q")
            for ko in range(8):
                proj_ko(ko, 0, xq_t, wq_sb, qch0)
            proj_bias(0, qch0, 0, qh_t)

            # K m=0 interleaved with Q m=1: K MMs are xk-DMA-paced, Q m=1
            # fills the arrival gaps so the PE never idles into a HAM window.
            _, _, kch0 = proj_chains(0, xk_t, wk_sb, tag="k")
            _, _, qch1 = proj_chains(1, xq_t, wq_sb, tag="q")
            for ko in range(8):
                proj_ko(ko, 0, xk_t, wk_sb, kch0)
                proj_ko(ko, 1, xq_t, wq_sb, qch1)
            # K m0 bias FIRST on the DVE queue: it gates QK(0)/exp(0);
            # Qm1 bias is only needed at the m=1 units (iter 40).
            proj_bias(0, kch0, 2, kh_t)
            proj_bias(1, qch1, 0, qh_t)

            # ---- attention (ACT-bound; deferred PE work fills the slack) ---
            # m=0 descending (big at-buffered unit first), m=1 ASCENDING so
            # the stream ends on the big (3,1) unit whose AVs/outproj(2)
            # overlap its own exps; only outproj(3) + one norm trail.
            units = [(qs, 0) for qs in (3, 2, 1, 0)] + \
                    [(qs, 1) for qs in (3, 2, 1, 0)]
            iters = []
            for u, (qs, m) in enumerate(units):
                nkb = 4 * qs + 4 if causal else NKB
                for kb in range(nkb):
                    iters.append((u, qs, m, kb, nkb))
            n_it = len(iters)

            st_of = {}
            at_of = {}
            cps_of = {}

            def emit_qk(i):
                u, qs, m, kb, nkb = iters[i]
                r = kb - 4 * qs
                qlo = r * P if (causal and r >= 0) else 0
                st = stp.tile([P, 2, QSUP], f32, tag="ps2",
                              name=f"st_{u}_{kb}")
                ksb, kof = kb // 4, (kb % 4) * P
                for h2 in range(2):
                    pl = slice(h2 * D, h2 * D + D)
                    nc.tensor.matmul(
                        st[:, h2, qlo:],
                        kh_t[m][ksb][pl, kof:kof + P],
                        qh_t[m][qs][pl, qlo:],
                        start=True, stop=True,
                    )
                st_of[i] = (st, qlo)

            def emit_exp_mask(i):
                u, qs, m, kb, nkb = iters[i]
                st, qlo = st_of.pop(i)
                at = attn.tile([P, 2, QSUP], bf16, tag="at",
                               name=f"at_{u}_{kb}")
                nc.scalar.activation(at[:, :, qlo:], st[:, :, qlo:],
                                     AF.Exp, scale=LN2)
                if causal and kb - 4 * qs >= 0:
                    nc.vector.tensor_mul(
                        at[:, :, qlo:qlo + P], at[:, :, qlo:qlo + P], cm_sb,
                    )
                at_of[i] = (at, qlo)

            # deferred queue: (fn, cost_ns, min_iter)
            deferred = []

            def emit_av(i):
                u, qs, m, kb, nkb = iters[i]
                at, qlo = at_of.pop(i)
                if kb == 0:
                    cps_of[u] = cpool.tile([P, 2, QSUP], f32, tag="cp2",
                                           name=f"cps_{u}")
                cps = cps_of[u]
                for h2 in range(2):
                    h = 2 * m + h2
                    nc.tensor.matmul(
                        cps[:, h2, qlo:],
                        vha[:, kb, h, :],
                        at[:, h2, qlo:],
                        start=(kb == 0), stop=(kb == nkb - 1),
                    )

            def emit_norm(u):
                qs, m = units[u]
                cps = cps_of.pop(u)
                for h2 in range(2):
                    sums = norm.tile([D, QSUP], f32, tag="sums",
                                     name=f"sums_{u}_{h2}")
                    if u == len(units) - 1:
                        # post-stream: ScalarE is idle after the last exp,
                        # take the sums copy off the DVE critical path
                        nc.scalar.activation(sums, cps[D:, h2, :], AF.Copy)
                    else:
                        nc.vector.tensor_copy(out=sums, in_=cps[D:, h2, :])
                    rec = norm.tile([D, QSUP], f32, tag="rec",
                                    name=f"rec_{u}_{h2}")
                    nc.vector.reciprocal_approx_fast(out=rec, in_=sums)
                    nc.vector.tensor_mul(
                        ctxT[h2 * D:(h2 + 1) * D, m,
                             qs * QSUP:(qs + 1) * QSUP],
                        cps[0:D, h2, :], rec)

            pso_state = {}

            def emit_chunk(qs, sb, j):
                # qs==0 (the tail outproj): two psos ping-pong + ACT
                # copies (ScalarE is idle after the last exp)
                if qs == 0:
                    key = "a" if (j & 1) == 0 else "b"
                    if key not in pso_state:
                        pso_state[key] = cpool.tile(
                            [P, 2, QSUP], f32, tag="cp2", name=f"pso0{key}")
                    pso = pso_state[key]
                else:
                    if qs not in pso_state:
                        pso_state[qs] = cpool.tile(
                            [P, 2, QSUP], f32, tag="cp2", name=f"pso_{qs}")
                    pso = pso_state[qs]
                for km in range(2):
                    for n2 in range(2):
                        nc.tensor.matmul(
                            pso[:, n2, :],
                            ctxT[:, km, sb * P:(sb + 1) * P],
                            wo_sb[:, km, n2 * QSUP:(n2 + 1) * QSUP],
                            start=(km == 0), stop=(km == 1),
                        )
                ot = osb.tile([P, 2, QSUP], bf16, tag="ot", name=f"ot_{sb}")
                if qs <= 1:
                    nc.scalar.activation(
                        ot.rearrange("p a b -> p (a b)"),
                        pso.rearrange("p a b -> p (a b)"),
                        AF.Copy,
                    )
                else:
                    nc.vector.tensor_copy(
                        out=ot.rearrange("p a b -> p (a b)"),
                        in_=pso.rearrange("p a b -> p (a b)"),
                    )
                nc.sync.dma_start(
                    out[sb * P:(sb + 1) * P, :],
                    ot.rearrange("p a b -> p (a b)"),
                )

            # --- pre-seed the deferred queue -------------------------------
            # Q m=1 then K m=1 projections (per-ko pop items + bias items),
            # with the V chains interleaved 2:1 between the K m=1 items so
            # both finish in time (kh[1] needed at iter 40, vha from the
            # first popped AV).
            pch_box = {}

            def pm1_ko(xt, w_sb, key, ko):
                def fn():
                    if key not in pch_box:
                        pch_box[key] = proj_chains(1, xt, w_sb, tag=key,
                                                   both_cpool=True)
                    _, _, ch = pch_box[key]
                    proj_ko(ko, 1, xt, w_sb, ch)
                return fn

            def pm1_bias(key, bcol, dst):
                def fn():
                    _, _, ch = pch_box[key]
                    proj_bias(1, ch, bcol, dst)
                return fn

            def v_chain(sb):
                def fn():
                    ps = cpool.tile([P, 2, QSUP], f32, tag="cp2",
                                    name=f"vp_{sb}")
                    for ko in range(8):
                        nc.tensor.matmul(
                            ps[:, 0, 0:DC],
                            xv_t[ko][:, sb * P:(sb + 1) * P],
                            wv_sb[:, ko, :],
                            start=(ko == 0), stop=(ko == 7),
                        )
                    nc.vector.tensor_copy(
                        vha[:, sb, :, 0:D],
                        ps[:, 0, 0:DC].rearrange("p (h d) -> p h d", h=HPC),
                    )
                return fn

            # FIFO: Km1 (needed by iter 40), then V chains (gated on the
            # xv DMA tail), then the in-loop AV/norm/outproj appends.
            for ko in range(8):
                deferred.append((pm1_ko(xk_t, wk_sb, "k1", ko), 900,
                                 1 + ko // 2))
            deferred.append((pm1_bias("k1", 2, kh_t), 100, 5))
            for sb in range(NKB):
                deferred.append((v_chain(sb), 950, 5 + sb // 2))
            # everything above MUST be emitted before the first m=1 QK is
            # emitted (iteration FORCE_BY), else that QK reads kh_t[1]
            # before its writes exist in the program -> no dep -> garbage
            n_seed = len(deferred)
            first_m1 = next(ix for ix, it in enumerate(iters) if it[2] == 1)
            FORCE_BY = first_m1 - 7

            # --- main attention loop ---------------------------------------
            credit = 0.0
            n_popped = 0
            emit_qk(0)
            for i in range(n_it):
                u, qs, m, kb, nkb = iters[i]
                r = kb - 4 * qs
                qlo = r * P if (causal and r >= 0) else 0
                cols = QSUP - qlo
                if i >= FORCE_BY:
                    while n_popped < n_seed:
                        fn, cost, mi = deferred.pop(0)
                        fn()
                        n_popped += 1
                emit_exp_mask(i)
                if i + 1 < n_it:
                    emit_qk(i + 1)

                # enqueue this iteration's AV (runs once popped)
                def av_fn(ii=i):
                    return emit_av(ii)
                deferred.append((av_fn, 2 * cols / 2.4, i + 1))
                if kb == nkb - 1:
                    def norm_fn(uu=u):
                        return emit_norm(uu)
                    deferred.append((norm_fn, 120, i + 1))
                    if m == 1:
                        for jj, sb in enumerate(range(4 * qs, 4 * qs + 4)):
                            def op_fn(q_=qs, s_=sb, j_=jj):
                                return emit_chunk(q_, s_, j_)
                            deferred.append((op_fn, 950, i + 2))

                # pops run BEFORE the 2-ahead QK: its st-slot wait (on
                # exp(i) completing) must not block them
                credit += (2 * cols * 0.8 + 295) - (cols / 2.4 + 100)
                credit = min(credit, 2600.0)
                while deferred:
                    fn, cost, mi = deferred[0]
                    if mi > i or cost > credit:
                        break
                    deferred.pop(0)
                    fn()
                    credit -= cost
                    n_popped += 1

            while deferred:
                fn, cost, mi = deferred.pop(0)
                fn()

    nc.finalize()
    return nc


def _get_nc(causal: bool):
    key = ("nc", causal)
    if key not in _CACHE:
        _CACHE[key] = _build_nc(causal)
    return _CACHE[key]


def _bf(a):
    return np.ascontiguousarray(a, dtype=np.float32).astype(BF16)


def _wperm(wT, nko):
    """[nko*128, M] -> [128, nko, M] so each SBUF partition's data is one
    contiguous run in DRAM (single DMA descriptor per partition)."""
    wT = np.asarray(wT, np.float32)
    m = wT.shape[1]
    return np.ascontiguousarray(
        wT.reshape(nko, P, m).transpose(1, 0, 2)).astype(BF16)


def kernel(q, k, v, mask, Wq, bq, Wk, bk, Wv, bv, Wo, bo):
    q = np.asarray(q, np.float32)
    k = np.asarray(k, np.float32)
    v = np.asarray(v, np.float32)
    mask = np.asarray(mask)
    Wq, bq = np.asarray(Wq, np.float32), np.asarray(bq, np.float32)
    Wk, bk = np.asarray(Wk, np.float32), np.asarray(bk, np.float32)
    Wv, bv = np.asarray(Wv, np.float32), np.asarray(bv, np.float32)
    Wo, bo = np.asarray(Wo, np.float32), np.asarray(bo, np.float32)

    m2 = mask.reshape(S, S) != 0
    if m2.all():
        causal = False
    else:
        tri = np.tril(np.ones((S, S), bool))
        assert (m2 == tri).all(), "only causal or all-ones masks supported"
        causal = True

    nc = _get_nc(causal)

    cm1 = np.asarray(
        np.arange(P)[:, None] <= np.arange(P)[None, :], np.float32
    ).astype(BF16)  # [k, q] keep-region of the diagonal 128-band
    cm = np.ascontiguousarray(
        np.broadcast_to(cm1[:, None, :], (P, 2, P))).astype(BF16)

    xT = {}
    for b in range(B):
        xT[("q", b)] = _bf(q[b].T)
        xT[("k", b)] = _bf(k[b].T)
        xT[("v", b)] = _bf(v[b].T)

    # log2(e)/sqrt(D) folded into Wq/bq: scores come out in the log2 domain
    qscale = LOG2E / SCALE
    in_maps = []
    for c in range(NCORES):
        b = c // 4
        rows = slice((c % 4) * DC, (c % 4) * DC + DC)
        bq_s = (bq[rows] * qscale).reshape(2, P).T
        bk_s = bk[rows].reshape(2, P).T
        in_maps.append({
            "xqT": xT[("q", b)],
            "xkT": xT[("k", b)],
            "xvT": xT[("v", b)],
            "wqT": _wperm(Wq[rows].T * qscale, 8),
            "wkT": _wperm(Wk[rows].T, 8),
            "wvT": _wperm(Wv[rows].T, 8),
            "woT": _wperm(Wo[:, rows].T, 2),
            "bqk": np.ascontiguousarray(
                np.concatenate([bq_s, bk_s], axis=1), np.float32),
            "cmask": cm,
        })

    res = run_bass_kernel_spmd(nc, in_maps, core_ids=list(range(NCORES)))
    LAST["exec_time_ns"] = res.exec_time_ns
    LAST["results"] = res

    host_bias = (bo + bv @ Wo.T).astype(np.float32)
    out = np.zeros((B, S, E), np.float32)
    for c in range(NCORES):
        out[c // 4] += res.results[c]["out"].astype(np.float32)
    out += host_bias
    return out
